# revision 40
# baseline (speedup 1.0000x reference)
"""3-layer GCN (CrystalGCN) on 8 TRN2 NeuronCores.

Strategy (graph/data parallel, nodes sharded by range):
  - 50000 nodes -> 6250/core (padded 6272 = 49 tiles of 128).
  - Edges (excl. self-loops) assigned to the core owning their dst.
  - Per layer l: z = (A_hat @ h) @ W + b  (associativity: aggregate first).
    * gather h[src] rows via gpsimd.dma_gather (bf16, sorted by src,
      lo/hi table split to fit int16 indices, 1024-idx calls),
    * self-loop term handled as a local matmul (no gather),
    * scatter-add via PE matmul: aggT[f, dst] += msgs[e, f].T @ S[e, dst],
      S one-hot * dinv[dst] built on DVE (bf16 iota is_equal + mult),
    * L1: dense matmul aggT.T @ W1, relu epilogue -> h1 [n, 256].
    * L2: z2T = W2sub.T @ agg chunks (feature-major), relu, then
      m3T = W3.T @ h2T locally (transform-first for layer 3!), transpose
      to m3 [n, 16] -> tiny AllGather (1.6MB vs 25.7MB).
    * L3: gather m3_pad (16 valid cols of 128) per edge, scatter to
      z3[dst, 16], batched log_softmax epilogue.
  - AllGather replicates h1 (25.7MB) and m3 (1.6MB) between layers.

Host preprocessing (numpy) builds index/metadata tensors; the device
kernel is static given the (fixed) edge distribution statistics.
"""
import numpy as np
import ml_dtypes

N = 50000
E = 800000
F_IN, F_HID, F_OUT = 128, 256, 10
F_OUT_P = 16
NCORES = 8
NSH = N // NCORES            # 6250
P = 128
NT = (NSH + P - 1) // P      # 49 node tiles per core
NSHP = NT * P                # 6272 padded shard rows
NROWS = NSHP * NCORES        # 50176 padded global rows
HI_OFF = 17408               # hi table = rows [17408, 50176), 32768 rows
LO_LIM = 32768
BATCH = 4                    # node tiles per gather pair
CALL_IDX = 1024              # idx per dma_gather call (SWDGE ring 1024)
SCRATCH = 16384              # dynamic_dma_scratch_size (1024 descriptors)
AGK = 1                      # row layout: 1 = core-major
AGR = NSHP // AGK
AG_DUMMY = True              # lead grouped collectives with a tiny pair

BF16 = ml_dtypes.bfloat16


def _wrap_idx16(vals):
    """dma_gather index layout: edge i -> [i%16, i//16], replicated to 8
    groups of 16 partitions (one copy per Q7 core)."""
    n = len(vals)
    assert n % 16 == 0
    blk = np.asarray(vals, dtype=np.int16).reshape(n // 16, 16).T
    return np.tile(blk, (8, 1))


def _preprocess(x, edge_index):
    """Build per-core gather/scatter metadata. Returns dict of host arrays."""
    x = np.asarray(x, dtype=np.float32)
    ei = np.asarray(edge_index, dtype=np.int64)
    # degree INCLUDES self-loops (reference semantics)
    dst_with_loops = np.concatenate([ei[1], np.arange(N, dtype=np.int64)])
    deg = np.bincount(dst_with_loops, minlength=N).astype(np.float32)
    dinv = np.where(deg > 0, 1.0 / np.sqrt(deg), 0.0).astype(np.float32)

    # streams exclude self-loops (they become local matmuls)
    src_all = ei[0]
    dst_all = ei[1]
    # pair-major global row layout: node (core c, local l) lives at
    # (l//AGR)*(NCORES*AGR) + c*AGR + (l%AGR)  [contiguous AllGather pairs]
    c_of = src_all // NSH
    l_of = src_all % NSH
    gidx_all = (l_of // AGR) * (NCORES * AGR) + c_of * AGR + (l_of % AGR)

    # x gather source: dinv-prescaled, pair-major layout, bf16
    x_pad = np.zeros((NROWS, F_IN), dtype=BF16)
    xs = (x * dinv[:, None]).astype(BF16)
    alln = np.arange(N, dtype=np.int64)
    gl = alln % NSH
    gall = (gl // AGR) * (NCORES * AGR) + (alln // NSH) * AGR + (gl % AGR)
    x_pad[gall] = xs
    xloc_s = []
    for c in range(NCORES):
        xl = np.zeros((NSHP, F_IN), dtype=BF16)
        xl[:NSH] = xs[c * NSH:(c + 1) * NSH]
        xloc_s.append(xl)

    core_of = dst_all // NSH
    tile_of = (dst_all % NSH) // P
    slot_of = (dst_all % NSH) % P

    order = np.lexsort((gidx_all, tile_of, core_of))
    g_sorted = gidx_all[order]
    slot_sorted = slot_of[order]
    dst_sorted = dst_all[order]
    key = core_of[order] * NT + tile_of[order]
    starts = np.searchsorted(key, np.arange(NCORES * NT))
    ends = np.searchsorted(key, np.arange(NCORES * NT), side="right")

    per = {}
    cnts = np.zeros((NCORES, NT), dtype=np.int64)
    lo_cap = np.zeros((NCORES, NT), dtype=np.int64)
    tl_need = np.zeros((NCORES, NT), dtype=np.int64)
    for c in range(NCORES):
        for t in range(NT):
            k = c * NT + t
            g = g_sorted[starts[k]:ends[k]]
            per[(c, t)] = (g, slot_sorted[starts[k]:ends[k]],
                           dst_sorted[starts[k]:ends[k]])
            cnts[c, t] = len(g)
            lo_cap[c, t] = np.searchsorted(g, LO_LIM)
            # edges beyond TL*P must have gidx >= HI_OFF
            tl_need[c, t] = np.searchsorted(g, HI_OFF)
    tl_min = int(np.ceil(tl_need.max() / P)) if tl_need.max() else 0
    tl_max = int(lo_cap.min() // P)
    assert tl_min <= tl_max, (tl_min, tl_max)
    TL = int(np.clip(8, tl_min, tl_max))
    # variable hi tile count per (core, tile); shared across cores for a
    # static SPMD program: use per-tile max over cores
    th_per = np.maximum(0, np.ceil((cnts - TL * P) / P)).astype(np.int64)
    TH_t = th_per.max(axis=0)          # [NT] shared tile structure
    TNT_t = TL + TH_t                  # edge tiles per dst tile (no self)
    tot_tiles = int(TNT_t.sum())

    batches = [list(range(i, min(i + BATCH, NT))) for i in range(0, NT, BATCH)]
    # meta column offset of tile t's edge-tile metadata
    moff = np.zeros(NT + 1, dtype=np.int64)
    moff[1:] = np.cumsum(TNT_t)
    # hi-stream tile offset within a batch
    hi_off_in_batch = {}
    for batch in batches:
        acc = 0
        for t in batch:
            hi_off_in_batch[t] = acc
            acc += int(TH_t[t])

    cores = []
    for c in range(NCORES):
        dslot = np.zeros((P, tot_tiles), dtype=np.float32)
        dinvd = np.zeros((P, tot_tiles), dtype=np.float32)
        lo_idx_parts = []
        hi_idx_parts = []
        for batch in batches:
            lo_stream = np.zeros(len(batch) * TL * P, dtype=np.int64)
            nhi_b = int(TH_t[batch].sum()) * P
            hi_stream = np.zeros(max(nhi_b, 16), dtype=np.int64)
            for bi, t in enumerate(batch):
                g, sl, dd = per[(c, t)]
                nlo = min(TL * P, len(g))
                glo, ghi = g[:nlo], g[nlo:]
                sllo, slhi = sl[:nlo], sl[nlo:]
                ddlo, ddhi = dd[:nlo], dd[nlo:]
                assert (len(glo) == 0 or glo.max() < LO_LIM)
                assert (len(ghi) == 0 or ghi.min() >= HI_OFF)
                lo_stream[bi * TL * P:bi * TL * P + nlo] = glo
                hb = hi_off_in_batch[t] * P
                nhi = len(ghi)
                assert nhi <= TH_t[t] * P
                hi_stream[hb:hb + nhi] = ghi - HI_OFF
                # hi dummies stay idx 0 (valid row, dinv 0)
                mlo = np.arange(nlo)
                dslot[mlo % P, moff[t] + mlo // P] = sllo
                dinvd[mlo % P, moff[t] + mlo // P] = dinv[ddlo]
                mhi = np.arange(nhi)
                dslot[mhi % P, moff[t] + TL + mhi // P] = slhi
                dinvd[mhi % P, moff[t] + TL + mhi // P] = dinv[ddhi]
            lo_idx_parts.append(_wrap_idx16(lo_stream))
            hi_idx_parts.append(_wrap_idx16(hi_stream))
        idx_lo = np.concatenate(lo_idx_parts, axis=1)
        idx_hi = np.concatenate(hi_idx_parts, axis=1)

        dinv_node = np.zeros((P, NT), dtype=np.float32)
        loc = np.arange(NSH)
        dinv_node[loc % P, loc // P] = dinv[c * NSH:(c + 1) * NSH]

        # meta (fp32 block): dslot | dinvd | dinv_node | selfslot
        selfslot = np.broadcast_to(np.arange(P, dtype=np.float32)[:, None],
                                   (P, 1))
        metaf = np.concatenate([dslot, dinvd, dinv_node, selfslot],
                               axis=1).astype(np.float32)
        # meta16 (bf16 block): iota | identity
        iota = np.broadcast_to(np.arange(P, dtype=np.float32), (P, P))
        ident = np.eye(P, dtype=np.float32)
        meta16 = np.concatenate([iota, ident], axis=1).astype(BF16)
        cores.append({"idx_lo": idx_lo, "idx_hi": idx_hi, "meta": metaf,
                      "meta16": meta16, "xloc": xloc_s[c]})

    return {
        "x_pad": x_pad, "cores": cores, "TL": TL,
        "TH_t": tuple(int(v) for v in TH_t),
        "batches": batches, "moff": moff,
        "hi_off_in_batch": hi_off_in_batch,
        "lo_cols": idx_lo.shape[1], "hi_cols": idx_hi.shape[1],
        "tot_tiles": tot_tiles,
    }


def _build_program(prep):
    import concourse.bass as bass
    from concourse import bacc
    import concourse.mybir as mybir
    from concourse.tile import TileContext
    from concourse.ap import AP

    dt = mybir.dt
    Alu = mybir.AluOpType
    Act = mybir.ActivationFunctionType

    TL = prep["TL"]
    TH_t = prep["TH_t"]
    batches = prep["batches"]
    moff = prep["moff"]
    hib = prep["hi_off_in_batch"]
    tot_tiles = prep["tot_tiles"]
    LO_COLS, HI_COLS = prep["lo_cols"], prep["hi_cols"]

    nc = bacc.Bacc(num_devices=NCORES, dynamic_dma_scratch_size=SCRATCH)
    x_pad = nc.dram_tensor("x_pad", [NROWS, F_IN], dt.bfloat16, kind="ExternalInput")
    xloc_d = nc.dram_tensor("xloc", [NSHP, F_IN], dt.bfloat16, kind="ExternalInput")
    idx_lo = nc.dram_tensor("idx_lo", [P, LO_COLS], dt.int16, kind="ExternalInput")
    idx_hi = nc.dram_tensor("idx_hi", [P, HI_COLS], dt.int16, kind="ExternalInput")
    meta = nc.dram_tensor("meta", [P, 2 * tot_tiles + NT + 1], dt.float32,
                          kind="ExternalInput")
    meta16 = nc.dram_tensor("meta16", [P, 256], dt.bfloat16, kind="ExternalInput")
    # wts: W1 [0:256] | W2sub fi0fo0 fi0fo1 fi1fo0 fi1fo1 [256:768] |
    #      W3c0 [768:784] | W3c1 [784:800]
    wts = nc.dram_tensor("wts", [P, 800], dt.bfloat16, kind="ExternalInput")
    # bias: b1row [0:256] | b3row [256:272] | b2Tcol0 [272] | b2Tcol1 [273]
    bias = nc.dram_tensor("bias", [P, 274], dt.float32, kind="ExternalInput")
    out_d = nc.dram_tensor("out", [NSHP, F_OUT_P], dt.float32, kind="ExternalOutput")

    with TileContext(nc) as tc:
        with tc.tile_pool(name="const", bufs=1) as cpool, \
             tc.tile_pool(name="msgs", bufs=2) as mpool, \
             tc.tile_pool(name="work", bufs=3) as wpool, \
             tc.tile_pool(name="big", bufs=1) as bigpool, \
             tc.tile_pool(name="ps", bufs=2, space="PSUM") as pspool, \
             tc.tile_pool(name="dram", bufs=1, space="DRAM") as dpool:

            # ordered by first use: lo idx + meta gate the first gathers/S
            idxlo_sb = cpool.tile([P, LO_COLS], dt.int16)
            nc.sync.dma_start(out=idxlo_sb[:], in_=idx_lo[:])
            meta_sb = cpool.tile([P, 2 * tot_tiles + NT + 1], dt.float32)
            nc.sync.dma_start(out=meta_sb[:], in_=meta[:])
            meta16_sb = cpool.tile([P, 256], dt.bfloat16)
            nc.sync.dma_start(out=meta16_sb[:], in_=meta16[:])
            idxhi_sb = cpool.tile([P, HI_COLS], dt.int16)
            nc.sync.dma_start(out=idxhi_sb[:], in_=idx_hi[:])
            wts_sb = cpool.tile([P, 800], dt.bfloat16)
            nc.sync.dma_start(out=wts_sb[:], in_=wts[:])
            bias_sb = cpool.tile([P, 274], dt.float32)
            nc.sync.dma_start(out=bias_sb[:], in_=bias[:])

            iota_ap = meta16_sb[:, 0:P]           # bf16
            ident_ap = meta16_sb[:, P:2 * P]      # bf16
            dslot0 = 0
            dinvd0 = tot_tiles
            dinvn0 = 2 * tot_tiles
            self0 = 2 * tot_tiles + NT

            h1_shard = dpool.tile([NSHP, F_HID], dt.bfloat16)
            m3_shard = dpool.tile([NSHP, F_OUT_P], dt.bfloat16)
            h1_full = dpool.tile([NROWS, F_HID], dt.bfloat16,
                                 addr_space="Shared")
            m3c_full = dpool.tile([NROWS, F_OUT_P], dt.bfloat16,
                                  addr_space="Shared")
            m3_pad = dpool.tile([NROWS + P, P], dt.bfloat16)
            if AG_DUMMY:
                dmy_in = dpool.tile([16, 16], dt.bfloat16, name="dmy_in")
                dmy_outs = [dpool.tile([16 * NCORES, 16], dt.bfloat16,
                                       addr_space="Shared", name=f"dmy_out{i}")
                            for i in range(2)]
                dmy_sb = cpool.tile([16, 16], dt.bfloat16)
                nc.vector.memset(dmy_sb[:], 0.0)
                nc.sync.dma_start(out=dmy_in[:], in_=dmy_sb[:])
                dmy_ctr = [0]

            w1 = wts_sb[:, 0:256]
            w2sub = [[wts_sb[:, 256 + (fi * 2 + fo) * P:256 + (fi * 2 + fo + 1) * P]
                      for fo in range(2)] for fi in range(2)]
            w3c = [wts_sb[:, 768:784], wts_sb[:, 784:800]]
            b1row = bias_sb[:, 0:256]
            b3row = bias_sb[:, 256:272]
            b2col = [bias_sb[:, 272:273], bias_sb[:, 273:274]]

            h1_big = bigpool.tile([P, NT, F_HID], dt.bfloat16)
            m3_big = bigpool.tile([P, NT, F_OUT_P], dt.bfloat16)
            out_big = bigpool.tile([P, NT, F_OUT_P], dt.float32)
            sums_sb = bigpool.tile([P, NT], dt.float32)

            def build_s(g, nt_for_self, name):
                """One-hot*dinv S tile [128e, 128dst] bf16 on DVE."""
                s_t = wpool.tile([P, P], dt.bfloat16, tag="s_t", bufs=12,
                                 name=name)
                if g is None:  # self tile: slot == partition index
                    sc1 = meta_sb[:, self0:self0 + 1]
                    sc2 = meta_sb[:, dinvn0 + nt_for_self:dinvn0 + nt_for_self + 1]
                else:
                    sc1 = meta_sb[:, dslot0 + g:dslot0 + g + 1]
                    sc2 = meta_sb[:, dinvd0 + g:dinvd0 + g + 1]
                nc.vector.tensor_scalar(out=s_t[:], in0=iota_ap,
                                        scalar1=sc1, scalar2=sc2,
                                        op0=Alu.is_equal, op1=Alu.mult)
                return s_t

            def gather_batch(l, b, batch, gsrc_lo, gsrc_hi, fe):
                """Issue gather calls for one batch; returns (msl, msh) tiles.
                fe = gathered row width (elements)."""
                nb = len(batch)
                nlo_t = nb * TL
                nhi_t = int(sum(TH_t[t] for t in batch))
                msl = mpool.tile([P, BATCH * TL, fe], dt.bfloat16,
                                 tag="msl", bufs=2, name=f"msl_{l}_{b}")
                msh = mpool.tile([P, max(int(sum(TH_t[t] for t in bb)) for bb in batches), fe],
                                 dt.bfloat16, tag="msh", bufs=2, name=f"msh_{l}_{b}")
                c0lo = batch[0] * TL * 8          # idx col offset (8 cols/tile)
                c0hi = sum(int(v) for v in TH_t[:batch[0]]) * 8
                step = CALL_IDX // P              # tiles per call
                for off in range(0, nlo_t, step):
                    ct = min(step, nlo_t - off)
                    nc.gpsimd.dma_gather(
                        out_ap=msl[:, off:off + ct, :],
                        in_ap=gsrc_lo,
                        idxs_ap=idxlo_sb[:, c0lo + off * 8:c0lo + (off + ct) * 8],
                        num_idxs=ct * P, num_idxs_reg=ct * P,
                        elem_size=fe)
                for off in range(0, nhi_t, step):
                    ct = min(step, nhi_t - off)
                    nc.gpsimd.dma_gather(
                        out_ap=msh[:, off:off + ct, :],
                        in_ap=gsrc_hi,
                        idxs_ap=idxhi_sb[:, c0hi + off * 8:c0hi + (off + ct) * 8],
                        num_idxs=ct * P, num_idxs_reg=ct * P,
                        elem_size=fe)
                return msl, msh

            def edge_tiles(batch, bi, nt, msl, msh, fe):
                """Yield (j_global_meta_col, msgs_ap) for tile nt."""
                for j in range(TL):
                    yield moff[nt] + j, msl[:, bi * TL + j, 0:fe]
                for j in range(int(TH_t[nt])):
                    yield moff[nt] + TL + j, msh[:, hib[nt] + j, 0:fe]

            # ---------------- Layer 1 ----------------
            for b, batch in enumerate(batches):
                nb = len(batch)
                msl, msh = gather_batch(1, b, batch,
                                        x_pad[0:LO_LIM, :],
                                        x_pad[HI_OFF:HI_OFF + LO_LIM, :], F_IN)
                xloc = mpool.tile([P, BATCH, F_IN], dt.bfloat16, tag="xloc",
                                  bufs=2, name=f"xloc_{b}")
                nc.sync.dma_start(
                    out=xloc[:, 0:nb, :],
                    in_=xloc_d[:].rearrange("(t p) f -> p t f", p=P)[
                        :, batch[0]:batch[0] + nb, :])
                for bi, nt in enumerate(batch):
                    aggps = pspool.tile([P, P], dt.float32, space="PSUM",
                                        tag="agg0", bufs=2, name=f"agg_1_{nt}")
                    first = True
                    for g, m_ap in edge_tiles(batch, bi, nt, msl, msh, F_IN):
                        s_t = build_s(g, None, f"s_1_{nt}_{g}")
                        nc.tensor.matmul(aggps[:], lhsT=m_ap, rhs=s_t[:],
                                         start=first, stop=False)
                        first = False
                    s_self = build_s(None, nt, f"s_1self_{nt}")
                    nc.tensor.matmul(aggps[:], lhsT=xloc[:, bi, :],
                                     rhs=s_self[:], start=first, stop=True)
                    aggsb = wpool.tile([P, P], dt.bfloat16, tag="aggsb",
                                       bufs=4, name=f"aggsb_1_{nt}")
                    nc.scalar.copy(out=aggsb[:], in_=aggps[:])
                    zps = pspool.tile([P, F_HID], dt.float32, space="PSUM",
                                      tag="z", bufs=3, name=f"z_1_{nt}")
                    nc.tensor.matmul(zps[:], lhsT=aggsb[:], rhs=w1,
                                     start=True, stop=True)
                    tmp = wpool.tile([P, F_HID], dt.float32, tag="tmp",
                                     bufs=3, name=f"tmp_1_{nt}")
                    nc.vector.tensor_tensor(out=tmp[:], in0=zps[:], in1=b1row,
                                            op=Alu.add)
                    nc.scalar.activation(
                        out=h1_big[:, nt, :], in_=tmp[:], func=Act.Relu,
                        scale=meta_sb[:, dinvn0 + nt:dinvn0 + nt + 1])
                    nc.sync.dma_start(
                        out=h1_shard[:].rearrange("(t p) f -> p t f", p=P)[:, nt, :],
                        in_=h1_big[:, nt, :])

            def allgather(shard, full):
                """AllGather shard -> full. With AG_DUMMY, emit one grouped
                instruction whose leading pair is a tiny (in, out) tensor
                pair ahead of the real one."""
                if not AG_DUMMY:
                    nc.gpsimd.collective_compute(
                        "AllGather", mybir.AluOpType.bypass,
                        replica_groups=[list(range(NCORES))],
                        ins=[shard[:].opt()], outs=[full[:].opt()])
                    return
                dmy_out = dmy_outs[dmy_ctr[0]]
                dmy_ctr[0] += 1
                ins_l = [nc.gpsimd.lower_ap(dmy_in[:].opt()),
                         nc.gpsimd.lower_ap(shard[:].opt())]
                outs_l = [nc.gpsimd.lower_ap(dmy_out[:].opt()),
                          nc.gpsimd.lower_ap(full[:].opt())]
                nc.gpsimd.add_instruction(
                    mybir.InstCollectiveCompute(
                        name=f"I-{nc.next_id()}",
                        kind="AllGather", op=mybir.AluOpType.bypass,
                        replica_groups=[list(range(NCORES))],
                        ins=ins_l, outs=outs_l,
                        unique_tensors="No", cc_dim="Partition"))
                nc.has_collectives = True

            allgather(h1_shard, h1_full)

            # ---------------- Layer 2 (+ local m3 = h2 @ W3) ----------------
            for b, batch in enumerate(batches):
                msl, msh = gather_batch(2, b, batch,
                                        h1_full[0:LO_LIM, :],
                                        h1_full[HI_OFF:HI_OFF + LO_LIM, :], F_HID)
                for bi, nt in enumerate(batch):
                    aggps = [pspool.tile([P, P], dt.float32, space="PSUM",
                                         tag=f"agg{fc}", bufs=2,
                                         name=f"agg_2_{nt}_{fc}")
                             for fc in range(2)]
                    first = True
                    for g, m_ap in edge_tiles(batch, bi, nt, msl, msh, F_HID):
                        s_t = build_s(g, None, f"s_2_{nt}_{g}")
                        for fc in range(2):
                            nc.tensor.matmul(aggps[fc][:],
                                             lhsT=m_ap[:, fc * P:(fc + 1) * P],
                                             rhs=s_t[:], start=first, stop=False)
                        first = False
                    s_self = build_s(None, nt, f"s_2self_{nt}")
                    for fc in range(2):
                        nc.tensor.matmul(aggps[fc][:],
                                         lhsT=h1_big[:, nt, fc * P:(fc + 1) * P],
                                         rhs=s_self[:], start=first, stop=True)
                    aggsb = []
                    for fc in range(2):
                        a = wpool.tile([P, P], dt.bfloat16, tag="aggsb",
                                       bufs=4, name=f"aggsb_2_{nt}_{fc}")
                        nc.scalar.copy(out=a[:], in_=aggps[fc][:])
                        aggsb.append(a)
                    # z2T[fo_c] = sum_fi W2sub[fi][fo_c].T @ agg[fi]  -> [fo, dst]
                    h2ts = []
                    for fo in range(2):
                        z2t = pspool.tile([P, P], dt.float32, space="PSUM",
                                          tag="z", bufs=3, name=f"z2t_{nt}_{fo}")
                        for fi in range(2):
                            nc.tensor.matmul(z2t[:], lhsT=w2sub[fi][fo],
                                             rhs=aggsb[fi][:],
                                             start=(fi == 0), stop=(fi == 1))
                        h2t = wpool.tile([P, P], dt.bfloat16, tag="h2t",
                                         bufs=3, name=f"h2t_{nt}_{fo}")
                        nc.scalar.activation(out=h2t[:], in_=z2t[:],
                                             func=Act.Relu, bias=b2col[fo])
                        h2ts.append(h2t)
                    # m3T [16, dst] = sum_fo W3c[fo].T @ h2t[fo]
                    m3ps = pspool.tile([P, P], dt.float32, space="PSUM",
                                       tag="z", bufs=3, name=f"m3ps_{nt}")
                    for fo in range(2):
                        nc.tensor.matmul(m3ps[0:F_OUT_P, :], lhsT=w3c[fo],
                                         rhs=h2ts[fo][:], start=(fo == 0),
                                         stop=(fo == 1))
                    m3sb = wpool.tile([F_OUT_P, P], dt.bfloat16, tag="m3sb",
                                      bufs=3, name=f"m3sb_{nt}")
                    nc.scalar.copy(out=m3sb[:], in_=m3ps[0:F_OUT_P, :])
                    m3tp = pspool.tile([P, F_OUT_P], dt.bfloat16, space="PSUM",
                                       tag="m3tp", bufs=1, name=f"m3tp_{nt}")
                    nc.tensor.transpose(m3tp[:], m3sb[:],
                                        ident_ap[0:F_OUT_P, 0:F_OUT_P])
                    # scale by dinv[dst] while moving PSUM->SBUF
                    nc.vector.tensor_scalar(
                        out=m3_big[:, nt, :], in0=m3tp[:],
                        scalar1=meta_sb[:, dinvn0 + nt:dinvn0 + nt + 1],
                        scalar2=None, op0=Alu.mult)
                    nc.sync.dma_start(
                        out=m3_shard[:].rearrange("(t p) f -> p t f", p=P)[:, nt, :],
                        in_=m3_big[:, nt, :])

            allgather(m3_shard, m3c_full)
            # expand compact m3 [.,16] into 256B-strided rows of m3_pad,
            # lo-window first so lo gathers can start before hi expansion
            nc.sync.dma_start(out=m3_pad[0:LO_LIM, 0:F_OUT_P],
                              in_=m3c_full[0:LO_LIM, :])
            nc.sync.dma_start(out=m3_pad[LO_LIM:NROWS, 0:F_OUT_P],
                              in_=m3c_full[LO_LIM:NROWS, :])

            # ---------------- Layer 3 ----------------
            for b, batch in enumerate(batches):
                msl, msh = gather_batch(3, b, batch,
                                        m3_pad[0:LO_LIM, :],
                                        m3_pad[HI_OFF:HI_OFF + LO_LIM, :], P)
                for bi, nt in enumerate(batch):
                    z3ps = pspool.tile([P, F_OUT_P], dt.float32, space="PSUM",
                                       tag="agg0", bufs=2, name=f"z3_{nt}")
                    first = True
                    for g, m_ap in edge_tiles(batch, bi, nt, msl, msh, P):
                        s_t = build_s(g, None, f"s_3_{nt}_{g}")
                        nc.tensor.matmul(z3ps[:], lhsT=s_t[:],
                                         rhs=m_ap[:, 0:F_OUT_P],
                                         start=first, stop=False)
                        first = False
                    s_self = build_s(None, nt, f"s_3self_{nt}")
                    nc.tensor.matmul(z3ps[:], lhsT=s_self[:],
                                     rhs=m3_big[:, nt, :], start=first,
                                     stop=True)
                    tmp = wpool.tile([P, F_OUT_P], dt.float32, tag="tmp",
                                     bufs=3, name=f"tmp_3_{nt}")
                    nc.vector.tensor_tensor(out=tmp[:], in0=z3ps[:], in1=b3row,
                                            op=Alu.add)
                    mx = wpool.tile([P, 1], dt.float32, tag="mx", bufs=3,
                                    name=f"mx_{nt}")
                    nc.vector.tensor_reduce(out=mx[:], in_=tmp[:, 0:F_OUT],
                                            axis=mybir.AxisListType.X,
                                            op=Alu.max, negate=True)
                    ex = wpool.tile([P, F_OUT], dt.float32, tag="ex", bufs=3,
                                    name=f"ex_{nt}")
                    nc.scalar.activation(out=ex[:], in_=tmp[:, 0:F_OUT],
                                         func=Act.Exp, bias=mx[:])
                    nc.vector.tensor_reduce(out=sums_sb[:, nt:nt + 1], in_=ex[:],
                                            axis=mybir.AxisListType.X,
                                            op=Alu.add)
                    # stash (x - max) for the batched log pass
                    nc.vector.tensor_scalar(
                        out=out_big[:, nt, :], in0=tmp[:], scalar1=mx[:],
                        scalar2=None, op0=Alu.add)

            # batched ln(sum) then subtract, in two groups to shrink the tail
            lns = bigpool.tile([P, NT], dt.float32)
            NH = NT // 2
            for lo_t, hi_t in ((0, NH), (NH, NT)):
                nc.scalar.activation(out=lns[:, lo_t:hi_t],
                                     in_=sums_sb[:, lo_t:hi_t], func=Act.Ln)
                for nt in range(lo_t, hi_t):
                    nc.vector.tensor_scalar(
                        out=out_big[:, nt, 0:F_OUT],
                        in0=out_big[:, nt, 0:F_OUT],
                        scalar1=lns[:, nt:nt + 1], scalar2=None,
                        op0=Alu.subtract)
                nc.sync.dma_start(
                    out=out_d[:].rearrange("(t p) f -> p t f", p=P)[
                        :, lo_t:hi_t, :],
                    in_=out_big[:, lo_t:hi_t, :])

    nc.finalize()
    return nc


_CACHE = {}


def kernel(x, edge_index, W1, b1, W2, b2, W3, b3):
    from concourse.bass_utils import run_bass_kernel_spmd

    prep = _preprocess(x, edge_index)

    key = (prep["TL"], prep["TH_t"])
    if key not in _CACHE:
        _CACHE[key] = _build_program(prep)
    nc = _CACHE[key]

    W1 = np.asarray(W1, np.float32)
    W2 = np.asarray(W2, np.float32)
    W3 = np.asarray(W3, np.float32)
    wts = np.zeros((P, 800), dtype=BF16)
    wts[:, 0:256] = W1.astype(BF16)
    for fi in range(2):
        for fo in range(2):
            wts[:, 256 + (fi * 2 + fo) * P:256 + (fi * 2 + fo + 1) * P] = \
                W2[fi * P:(fi + 1) * P, fo * P:(fo + 1) * P].astype(BF16)
    wts[:, 768:778] = W3[0:128].astype(BF16)
    wts[:, 784:794] = W3[128:256].astype(BF16)
    bias = np.zeros((P, 274), dtype=np.float32)
    bias[:, 0:256] = np.asarray(b1, np.float32)[None, :]
    bias[:, 256:266] = np.asarray(b3, np.float32)[None, :]
    b2 = np.asarray(b2, np.float32)
    bias[:, 272] = b2[0:128]
    bias[:, 273] = b2[128:256]

    in_maps = []
    for c in range(NCORES):
        m = dict(prep["cores"][c])
        m["x_pad"] = prep["x_pad"]
        m["wts"] = wts
        m["bias"] = bias
        in_maps.append(m)

    res = run_bass_kernel_spmd(nc, in_maps, core_ids=list(range(NCORES)))
    out = np.zeros((N, F_OUT), dtype=np.float32)
    for c in range(NCORES):
        out[c * NSH:(c + 1) * NSH] = res.results[c]["out"][:NSH, :F_OUT]
    return out
